# revision 9
# baseline (speedup 1.0000x reference)
"""Swin-style windowed attention on 8 TRN2 NeuronCores.

Data-parallel over windows: core i handles windows [64i, 64i+64).
v3: minimal elementwise instruction count.

Per pair iteration i (windows w0=2i, w1=2i+1):
  back1(i-2): Z_all = U_all * R33 (one DVE mul), 3 zt-regroup SBUF DMAs
  back2(i-3): proj (4 MMs from zt), y copy (ACT), out DMA
  front(i):
    qk-proj pair-batched -> qT/kT [96, 392]; per-ft copy on DVE
    per window w:
      v-proj into one PSUM bank (2 regions); one merged DVE copy
      S^T 3-way row-packed -> 3 PSUM banks; exp (1 ACT instr per half)
      P = E * EBM (1 DVE mul per half)
      PV into O2 [98, 2x512] (2 banks, both halves)
      U_all <- O2: 2 merged copies (ACT rows 0:33, DVE rows 64:97)
      recip chain: gpsimd transpose DMA -> DVE recip -> gpsimd DMA back
                   -> sync broadcast DMA R33 [33, 1176]
Host: folds scale into w_qkv, builds EBM=exp(bias+mask), packs xa pairs,
permutes w_proj rows to (h0,h2,h1 | h3,h5,h4), adds b_proj at the end.
"""

import numpy as np
import ml_dtypes

import concourse.bass as bass
import concourse.mybir as mybir
import concourse.tile as tile
from concourse import bacc
from concourse import bass_utils
from concourse.bass import AP

BF16 = mybir.dt.bfloat16
F32 = mybir.dt.float32
NPBF16 = ml_dtypes.bfloat16

B, N, C, H, HD, NG = 512, 196, 192, 6, 32, 64
NCORES = 8
WPC = B // NCORES  # 64 windows per core
NPAIR = WPC // 2   # 32 pair iterations
MT = 98            # m-tile size, 2 tiles cover N=196

_CACHE = {}


def _build_nc():
    nc = bacc.Bacc("TRN2", target_bir_lowering=False, debug=False,
                   enable_asserts=False)

    xa_d = nc.dram_tensor("xa", [NPAIR, 97, 784], BF16, kind="ExternalInput").ap()
    ebm_d = nc.dram_tensor("ebm", [WPC, 98, 6 * 392], BF16, kind="ExternalInput").ap()
    wqk_d = nc.dram_tensor("wqk", [2, 96, 384], BF16, kind="ExternalInput").ap()
    wv_d = nc.dram_tensor("wv", [2, 97, 198], BF16, kind="ExternalInput").ap()
    wp_d = nc.dram_tensor("wp", [2, 96, 192], BF16, kind="ExternalInput").ap()
    out_d = nc.dram_tensor("out", [WPC, N, C], F32, kind="ExternalOutput").ap()

    with tile.TileContext(nc) as tc:
        with (
            tc.tile_pool(name="static", bufs=1) as static_pool,
            tc.tile_pool(name="xa", bufs=3) as xa_pool,
            tc.tile_pool(name="ebm", bufs=3) as ebm_pool,
            tc.tile_pool(name="qk", bufs=3) as qk_pool,
            tc.tile_pool(name="vaug", bufs=4) as vaug_pool,
            tc.tile_pool(name="ep", bufs=3) as ep_pool,
            tc.tile_pool(name="ua", bufs=8) as ua_pool,
            tc.tile_pool(name="za", bufs=8) as za_pool,
            tc.tile_pool(name="zt", bufs=8) as zt_pool,
            tc.tile_pool(name="ysb", bufs=3) as y_pool,
            tc.tile_pool(name="rr", bufs=8) as rr_pool,
            tc.tile_pool(name="spsum", bufs=1, space="PSUM") as s_psum,
            tc.tile_pool(name="opsum", bufs=1, space="PSUM") as o_psum,
            tc.tile_pool(name="ppsum", bufs=3, space="PSUM") as p_psum,
        ):
            # static weights
            wqk_t = []
            for kt in range(2):
                t = static_pool.tile([96, 384], BF16, tag=f"wqk{kt}")
                nc.sync.dma_start(t[:, :], wqk_d[kt])
                wqk_t.append(t)
            wv_t = []
            for kt in range(2):
                t = static_pool.tile([97, 198], BF16, tag=f"wv{kt}")
                nc.sync.dma_start(t[:, :], wv_d[kt])
                wv_t.append(t)
            wp_t = []
            for kt in range(2):
                t = static_pool.tile([96, 192], BF16, tag=f"wp{kt}")
                nc.sync.dma_start(t[:, :], wp_d[kt])
                wp_t.append(t)

            stage1 = []  # awaiting Z/regroup (back1)
            stage2 = []  # awaiting proj/out (back2)

            def back1(st):
                # Z_all = U_all * R33 (row 32 = s*r junk, unused)
                U_all, R33 = st["U"], st["R"]
                Z_all = za_pool.tile([33, 1176], BF16, tag="za")
                nc.vector.tensor_mul(Z_all[:, :], U_all[:, :], R33[:, :])
                # regroup into zt [96, 392]: rows 32u+d, cols hf*196+n
                # u-order heads: hf0 (h0, h2, h1), hf1 (h3, h5, h4)
                zt = zt_pool.tile([96, 392], BF16, tag="zt")
                zsrc = Z_all[0:32, :].rearrange("p (h x) -> p h x", h=2)
                for u in range(3):
                    zdst = zt[32 * u:32 * u + 32, :].rearrange(
                        "p (h x) -> p h x", h=2)
                    nc.gpsimd.dma_start(
                        zdst, zsrc[:, :, u * 196:(u + 1) * 196])
                st["zt"] = zt

            def back2(st):
                zt, w = st["zt"], st["w"]
                yp = p_psum.tile([128, 512], F32, tag="ps2")
                for nt in range(2):
                    for kt in range(2):
                        nc.tensor.matmul(
                            yp[0:98, nt * 256:nt * 256 + 192],
                            zt[:, kt * 196 + nt * 98: kt * 196 + nt * 98 + 98],
                            wp_t[kt][:, :],
                            start=(kt == 0), stop=(kt == 1),
                        )
                y_sb = y_pool.tile([98, 384], F32, tag="ysb")
                ysrc = yp[0:98, 0:512].rearrange("p (t x) -> p t x", t=2)[:, :, 0:192]
                ydst = y_sb[:, :].rearrange("p (t x) -> p t x", t=2)
                nc.scalar.copy(ydst, ysrc)
                nc.sync.dma_start(
                    out_d[w, :, :].rearrange("(t p) x -> p t x", t=2),
                    ydst)

            for it in range(NPAIR):
                if len(stage1) > 1:
                    for st in stage1.pop(0):
                        back1(st)
                if len(stage2) > 2:
                    for st in stage2.pop(0):
                        back2(st)

                # ---- pair input DMA ----
                xa = xa_pool.tile([97, 784], BF16, tag="xa")
                nc.sync.dma_start(xa[:, :], xa_d[it])

                # ---- qk projection, pair-batched: 4 ft x 2 kt, 392 cols ----
                # ft0=q h0-2, ft1=q h3-5, ft2=k h0-2, ft3=k h3-5
                qk_sb = []
                for ft in range(4):
                    ps = p_psum.tile([128, 512], F32, tag="ps2")
                    for kt in range(2):
                        nc.tensor.matmul(
                            ps[0:96, 0:392],
                            wqk_t[kt][:, ft * 96:(ft + 1) * 96],
                            xa[0:96, :].rearrange("p (w k j) -> p w k j",
                                                  w=2, k=2)[:, :, kt, :],
                            start=(kt == 0), stop=(kt == 1),
                        )
                    t = qk_pool.tile([96, 392], BF16, tag=f"qk{ft}")
                    nc.vector.tensor_copy(t[:, :], ps[0:96, 0:392])
                    qk_sb.append(t)
                qT_A, qT_B, kT_A, kT_B = qk_sb

                st_pair = []
                for wi in range(2):
                    w = 2 * it + wi
                    ebm = ebm_pool.tile([98, 6 * 392], BF16, tag="ebm")
                    nc.sync.dma_start(ebm[:, :], ebm_d[w])

                    # ---- v projection into one bank: mt0 @ 0, mt1 @ 256 ----
                    ps = p_psum.tile([128, 512], F32, tag="ps2")
                    for mt in range(2):
                        base = wi * 392 + mt * 98
                        for kt in range(2):
                            nc.tensor.matmul(
                                ps[0:98, mt * 256:mt * 256 + 198],
                                xa[0:97, base + kt * 196: base + kt * 196 + 98],
                                wv_t[kt][:, :],
                                start=(kt == 0), stop=(kt == 1),
                            )
                    va = vaug_pool.tile([98, 396], BF16, tag="vaug")
                    nc.vector.tensor_copy(
                        va[:, :].rearrange("p (t x) -> p t x", t=2),
                        ps[0:98, 0:512].rearrange("p (t x) -> p t x", t=2)[:, :, 0:198])

                    # ---- per half: QK^T -> exp -> mul -> PV ----
                    U_all = ua_pool.tile([33, 1176], BF16, tag="ua")
                    P = ep_pool.tile([98, 6 * 392], BF16, tag="P")
                    O2 = o_psum.tile([98, 1024], F32, tag="O2")
                    for hf in range(2):
                        kTh, qTh = (kT_A, qT_A) if hf == 0 else (kT_B, qT_B)
                        S = s_psum.tile([98, 3 * 512], F32, tag="S")
                        for hl in range(3):
                            for mt in range(2):
                                nc.tensor.matmul(
                                    S[:, hl * 512 + mt * 196: hl * 512 + (mt + 1) * 196],
                                    kTh[32 * hl:32 * hl + 32,
                                        wi * 196 + mt * 98: wi * 196 + mt * 98 + 98],
                                    qTh[32 * hl:32 * hl + 32,
                                        wi * 196:(wi + 1) * 196],
                                    start=True, stop=True,
                                    tile_position=(32 * hl, 0),
                                )
                        # exp across the 3 banks in one ACT instr
                        E = ep_pool.tile([98, 3 * 392], BF16, tag="E")
                        S3 = S[:, :].rearrange("p (h x) -> p h x", h=3)[:, :, 0:392]
                        E3 = E[:, :].rearrange("p (h x) -> p h x", h=3)
                        nc.scalar.activation(E3, S3, mybir.ActivationFunctionType.Exp)
                        nc.vector.tensor_mul(
                            P[:, hf * 1176:(hf + 1) * 1176], E[:, :],
                            ebm[:, hf * 1176:(hf + 1) * 1176])

                        # PV into O2 cols hf*512: l0 rows0-32 @0:196,
                        # l1 rows64-96 @0:196, l2 rows0-32 @196:392
                        for hl in range(3):
                            h = 3 * hf + hl
                            row = 64 if hl == 1 else 0
                            fo = hf * 512 + (196 if hl == 2 else 0)
                            for mt in range(2):
                                nc.tensor.matmul(
                                    O2[row:row + 33, fo:fo + 196],
                                    va[:, mt * 198 + 33 * h: mt * 198 + 33 * h + 33],
                                    P[:, h * 392 + mt * 196: h * 392 + (mt + 1) * 196],
                                    start=(mt == 0), stop=(mt == 1),
                                )

                    # ---- merged U copies: 2 instrs for both halves ----
                    udst = U_all[:, :].rearrange("p (h x) -> p h x", h=2)
                    osrc = O2[0:33, :].rearrange("p (h x) -> p h x", h=2)
                    nc.scalar.copy(udst[:, :, 0:392], osrc[:, :, 0:392])
                    osrc2 = O2[64:97, :].rearrange("p (h x) -> p h x", h=2)
                    nc.vector.tensor_copy(udst[:, :, 392:588], osrc2[:, :, 0:196])

                    # ---- reciprocal chain ----
                    s_t = rr_pool.tile([98, 12], F32, tag="st")
                    nc.gpsimd.dma_start(s_t[:, :], U_all[32:33, :])
                    r_t = rr_pool.tile([98, 12], BF16, tag="rt")
                    with nc.allow_low_precision(reason="softmax recip; rel_err gate 2e-2"):
                        nc.vector.reciprocal(r_t[:, :], s_t[:, :])
                    r_row = rr_pool.tile([1, 1176], BF16, tag="rrow")
                    nc.gpsimd.dma_start(r_row[0:1, :], r_t[:, :])
                    R33 = rr_pool.tile([33, 1176], BF16, tag="R33")
                    nc.sync.dma_start(
                        R33[:, :],
                        r_row[0:1, :].unsqueeze(1).broadcast_to([1, 33, 1176]))
                    st_pair.append({"U": U_all, "R": R33, "w": w})

                stage1.append(st_pair)
                stage2.append(st_pair)

            while stage2:
                if stage1:
                    for st in stage1.pop(0):
                        back1(st)
                for st in stage2.pop(0):
                    back2(st)
    nc.compile()
    return nc


def _host_precompute(x, w_qkv, w_proj, bias_table, mask, rel_index):
    scale = HD ** (-0.5)
    wq = np.array(w_qkv, np.float32).copy()
    wq[0:C] *= scale  # fold softmax scale into q weights

    # xa[w, p, kt*196 + j] = x[w, j, kt*96 + p]; row 96: kt0->0, kt1->1
    xT = np.ascontiguousarray(np.transpose(np.asarray(x, np.float32), (0, 2, 1)))
    xa = np.zeros((B, 97, 392), np.float32)
    xa[:, 0:96, 0:196] = xT[:, 0:96]
    xa[:, 0:96, 196:392] = xT[:, 96:192]
    xa[:, 96, 196:392] = 1.0
    # pack window pairs: [B/2, 97, 784] = (w0 | w1)
    xa = np.ascontiguousarray(
        xa.reshape(B // 2, 2, 97, 392).transpose(0, 2, 1, 3).reshape(B // 2, 97, 784))

    # wqk[kt, p, f] = wq[f, kt*96+p]  (f < 384: q then k features)
    wqkT = wq[0:384].T  # [192, 384]
    wqk = np.stack([wqkT[0:96], wqkT[96:192]])

    # wv[kt, p, 33h+d] = wq[384+32h+d, kt*96+p]; ones row kt1 p=96
    wv = np.zeros((2, 97, 198), np.float32)
    wvT = wq[384:576].T  # [192, 192] [c, (h,d)]
    for h in range(H):
        wv[0, 0:96, 33 * h: 33 * h + 32] = wvT[0:96, 32 * h: 32 * h + 32]
        wv[1, 0:96, 33 * h: 33 * h + 32] = wvT[96:192, 32 * h: 32 * h + 32]
        wv[1, 96, 33 * h + 32] = 1.0

    # wp[kt]: rows in zt u-order (h0,h2,h1 | h3,h5,h4)
    wpT = np.asarray(w_proj, np.float32).T  # [c, c']
    perm = np.r_[0:32, 64:96, 32:64]
    wp = np.stack([wpT[0:96][perm], wpT[96:192][perm]])

    # EBM[w, p, h*392 + mt*196 + n] = exp(bias[n, m, h] + mask[w, n, m]),
    # m = mt*98 + p
    bias = np.asarray(bias_table, np.float32)[np.asarray(rel_index).reshape(-1)]
    bias = bias.reshape(N, N, H)  # [n, m, h]
    biasT = np.transpose(bias, (2, 1, 0))  # [h, m, n]
    maskT = np.transpose(np.asarray(mask, np.float32), (0, 2, 1))  # [g, m, n]
    ebm = np.exp(biasT[None] + maskT[:, None])  # [g, h, m, n]
    ebm = ebm.reshape(NG, H, 2, MT, N).transpose(0, 3, 1, 2, 4)
    ebm = np.ascontiguousarray(ebm.reshape(NG, MT, H * 392))

    return (xa.astype(NPBF16), wqk.astype(NPBF16), wv.astype(NPBF16),
            wp.astype(NPBF16), ebm.astype(NPBF16))


def kernel(x, w_qkv, w_proj, b_proj, bias_table, mask, rel_index):
    xa, wqk, wv, wp, ebm = _host_precompute(
        x, w_qkv, w_proj, bias_table, mask, rel_index)

    if "nc" not in _CACHE:
        _CACHE["nc"] = _build_nc()
    nc = _CACHE["nc"]

    in_maps = []
    for c in range(NCORES):
        in_maps.append({
            "xa": np.ascontiguousarray(xa[c * NPAIR:(c + 1) * NPAIR]),
            "ebm": ebm,  # window w on core uses mask (64c+w) % 64 = w
            "wqk": wqk, "wv": wv, "wp": wp,
        })

    res = bass_utils.run_bass_kernel_spmd(nc, in_maps, core_ids=list(range(NCORES)))
    out = np.concatenate([res.results[c]["out"] for c in range(NCORES)], axis=0)
    out = out.astype(np.float32) + np.asarray(b_proj, np.float32)[None, None, :]
    return out


# revision 11
# speedup vs baseline: 1.0692x; 1.0692x over previous
"""Swin-style windowed attention on 8 TRN2 NeuronCores.

Data-parallel over windows: core i handles windows [64i, 64i+64).
v4: interleaved emission order so no PE matmul waits at the queue head
on the exp->P chain; software-pipelined qkproj (one pair ahead).

Per pair iteration i (w0=2i, w1=2i+1; ow* = pair i-3's windows):
  back1(i-2): Z_all = U_all * R33 (one DVE mul), 3 zt-regroup DMAs
  vproj(w0) | S(w0,h0)+exp+P | proj/y/out(ow0) | S(w0,h1)+exp+P |
  vproj(w1) | PV(w0,h0) | PV(w0,h1) | U(w0)+recip(w0) |
  S(w1,h0)+exp+P | proj/y/out(ow1) | S(w1,h1)+exp+P |
  qkproj(pair i+1) | PV(w1,h0) | PV(w1,h1) | U(w1)+recip(w1)
Host: folds scale into w_qkv, builds EBM=exp(bias+mask), packs xa pairs,
permutes w_proj rows to (h0,h2,h1 | h3,h5,h4), adds b_proj at the end.
"""

import numpy as np
import ml_dtypes

import concourse.bass as bass
import concourse.mybir as mybir
import concourse.tile as tile
from concourse import bacc
from concourse import bass_utils
from concourse.bass import AP

BF16 = mybir.dt.bfloat16
F32 = mybir.dt.float32
NPBF16 = ml_dtypes.bfloat16

B, N, C, H, HD, NG = 512, 196, 192, 6, 32, 64
NCORES = 8
WPC = B // NCORES  # 64 windows per core
NPAIR = WPC // 2   # 32 pair iterations
MT = 98            # m-tile size, 2 tiles cover N=196

_CACHE = {}


def _build_nc():
    nc = bacc.Bacc("TRN2", target_bir_lowering=False, debug=False,
                   enable_asserts=False)

    xa_d = nc.dram_tensor("xa", [NPAIR, 97, 784], BF16, kind="ExternalInput").ap()
    ebm_d = nc.dram_tensor("ebm", [WPC, 98, 6 * 392], BF16, kind="ExternalInput").ap()
    wqk_d = nc.dram_tensor("wqk", [2, 96, 384], BF16, kind="ExternalInput").ap()
    wv_d = nc.dram_tensor("wv", [2, 97, 198], BF16, kind="ExternalInput").ap()
    wp_d = nc.dram_tensor("wp", [2, 96, 192], BF16, kind="ExternalInput").ap()
    out_d = nc.dram_tensor("out", [WPC, N, C], F32, kind="ExternalOutput").ap()

    with tile.TileContext(nc) as tc:
        with (
            tc.tile_pool(name="static", bufs=1) as static_pool,
            tc.tile_pool(name="xa", bufs=3) as xa_pool,
            tc.tile_pool(name="ebm", bufs=3) as ebm_pool,
            tc.tile_pool(name="qk", bufs=3) as qk_pool,
            tc.tile_pool(name="vaug", bufs=4) as vaug_pool,
            tc.tile_pool(name="ep", bufs=3) as ep_pool,
            tc.tile_pool(name="ua", bufs=8) as ua_pool,
            tc.tile_pool(name="za", bufs=8) as za_pool,
            tc.tile_pool(name="zt", bufs=8) as zt_pool,
            tc.tile_pool(name="ysb", bufs=3) as y_pool,
            tc.tile_pool(name="rr", bufs=8) as rr_pool,
            tc.tile_pool(name="spsum", bufs=1, space="PSUM") as s_psum,
            tc.tile_pool(name="opsum", bufs=1, space="PSUM") as o_psum,
            tc.tile_pool(name="ppsum", bufs=3, space="PSUM") as p_psum,
        ):
            # static weights
            wqk_t = []
            for kt in range(2):
                t = static_pool.tile([96, 384], BF16, tag=f"wqk{kt}")
                nc.sync.dma_start(t[:, :], wqk_d[kt])
                wqk_t.append(t)
            wv_t = []
            for kt in range(2):
                t = static_pool.tile([97, 198], BF16, tag=f"wv{kt}")
                nc.sync.dma_start(t[:, :], wv_d[kt])
                wv_t.append(t)
            wp_t = []
            for kt in range(2):
                t = static_pool.tile([96, 192], BF16, tag=f"wp{kt}")
                nc.sync.dma_start(t[:, :], wp_d[kt])
                wp_t.append(t)

            pairs = {}   # pair idx -> {"xa": tile, "qk": [4 tiles]}
            stage1 = []  # pairs awaiting Z/regroup
            stage2 = []  # windows awaiting proj/out

            def emit_xa(p):
                xa = xa_pool.tile([97, 784], BF16, tag="xa")
                nc.sync.dma_start(xa[:, :], xa_d[p])
                pairs[p] = {"xa": xa}

            def emit_qkproj(p):
                # ft0=q h0-2, ft1=q h3-5, ft2=k h0-2, ft3=k h3-5
                xa = pairs[p]["xa"]
                qk_sb = []
                for ft in range(4):
                    ps = p_psum.tile([128, 512], F32, tag="ps2")
                    for kt in range(2):
                        nc.tensor.matmul(
                            ps[0:96, 0:392],
                            wqk_t[kt][:, ft * 96:(ft + 1) * 96],
                            xa[0:96, :].rearrange("p (w k j) -> p w k j",
                                                  w=2, k=2)[:, :, kt, :],
                            start=(kt == 0), stop=(kt == 1),
                        )
                    t = qk_pool.tile([96, 392], BF16, tag=f"qk{ft}")
                    nc.vector.tensor_copy(t[:, :], ps[0:96, 0:392])
                    qk_sb.append(t)
                pairs[p]["qk"] = qk_sb

            def emit_vproj(p, wi):
                xa = pairs[p]["xa"]
                ebm = ebm_pool.tile([98, 6 * 392], BF16, tag="ebm")
                nc.sync.dma_start(ebm[:, :], ebm_d[2 * p + wi])
                ps = p_psum.tile([128, 512], F32, tag="ps2")
                for mt in range(2):
                    base = wi * 392 + mt * 98
                    for kt in range(2):
                        nc.tensor.matmul(
                            ps[0:98, mt * 256:mt * 256 + 198],
                            xa[0:97, base + kt * 196: base + kt * 196 + 98],
                            wv_t[kt][:, :],
                            start=(kt == 0), stop=(kt == 1),
                        )
                va = vaug_pool.tile([98, 396], BF16, tag="vaug")
                nc.vector.tensor_copy(
                    va[:, :].rearrange("p (t x) -> p t x", t=2),
                    ps[0:98, 0:512].rearrange("p (t x) -> p t x", t=2)[:, :, 0:198])
                return {"va": va, "ebm": ebm, "w": 2 * p + wi, "p": p, "wi": wi}

            def emit_s_exp(wst, hf):
                p, wi = wst["p"], wst["wi"]
                qk = pairs[p]["qk"]
                kTh, qTh = (qk[2], qk[0]) if hf == 0 else (qk[3], qk[1])
                S = s_psum.tile([98, 3 * 512], F32, tag="S")
                for hl in range(3):
                    for mt in range(2):
                        nc.tensor.matmul(
                            S[:, hl * 512 + mt * 196: hl * 512 + (mt + 1) * 196],
                            kTh[32 * hl:32 * hl + 32,
                                wi * 196 + mt * 98: wi * 196 + mt * 98 + 98],
                            qTh[32 * hl:32 * hl + 32, wi * 196:(wi + 1) * 196],
                            start=True, stop=True,
                            tile_position=(32 * hl, 0),
                        )
                if hf == 0:
                    wst["P"] = ep_pool.tile([98, 6 * 392], BF16, tag="P", name="P")
                E = ep_pool.tile([98, 3 * 392], BF16, tag="E")
                S3 = S[:, :].rearrange("p (h x) -> p h x", h=3)[:, :, 0:392]
                E3 = E[:, :].rearrange("p (h x) -> p h x", h=3)
                nc.scalar.activation(E3, S3, mybir.ActivationFunctionType.Exp)
                nc.vector.tensor_mul(
                    wst["P"][:, hf * 1176:(hf + 1) * 1176], E[:, :],
                    wst["ebm"][:, hf * 1176:(hf + 1) * 1176])

            def emit_pv(wst, hf):
                if hf == 0:
                    wst["O2"] = o_psum.tile([98, 1024], F32, tag="O2", name="O2")
                O2, va, P = wst["O2"], wst["va"], wst["P"]
                for hl in range(3):
                    h = 3 * hf + hl
                    row = 64 if hl == 1 else 0
                    fo = hf * 512 + (196 if hl == 2 else 0)
                    for mt in range(2):
                        nc.tensor.matmul(
                            O2[row:row + 33, fo:fo + 196],
                            va[:, mt * 198 + 33 * h: mt * 198 + 33 * h + 33],
                            P[:, h * 392 + mt * 196: h * 392 + (mt + 1) * 196],
                            start=(mt == 0), stop=(mt == 1),
                        )

            def emit_u_recip(wst):
                O2 = wst["O2"]
                U_all = ua_pool.tile([33, 1176], BF16, tag="ua")
                udst = U_all[:, :].rearrange("p (h x) -> p h x", h=2)
                osrc = O2[0:33, :].rearrange("p (h x) -> p h x", h=2)
                nc.scalar.copy(udst[:, :, 0:392], osrc[:, :, 0:392])
                osrc2 = O2[64:97, :].rearrange("p (h x) -> p h x", h=2)
                nc.vector.tensor_copy(udst[:, :, 392:588], osrc2[:, :, 0:196])

                s_t = rr_pool.tile([98, 12], F32, tag="st")
                nc.gpsimd.dma_start(s_t[:, :], U_all[32:33, :])
                r_t = rr_pool.tile([98, 12], BF16, tag="rt")
                with nc.allow_low_precision(reason="softmax recip; rel_err gate 2e-2"):
                    nc.vector.reciprocal(r_t[:, :], s_t[:, :])
                r_row = rr_pool.tile([1, 1176], BF16, tag="rrow")
                nc.gpsimd.dma_start(r_row[0:1, :], r_t[:, :])
                R33 = rr_pool.tile([33, 1176], BF16, tag="R33")
                nc.sync.dma_start(
                    R33[:, :],
                    r_row[0:1, :].unsqueeze(1).broadcast_to([1, 33, 1176]))
                wst["U"], wst["R"] = U_all, R33

            def back1(wst):
                # Z_all = U_all * R33 (row 32 = s*r junk, unused), then
                # regroup to zt [96, 392]: rows 32u+d, u-order (h0,h2,h1)
                Z_all = za_pool.tile([33, 1176], BF16, tag="za")
                nc.vector.tensor_mul(Z_all[:, :], wst["U"][:, :], wst["R"][:, :])
                zt = zt_pool.tile([96, 392], BF16, tag="zt")
                zsrc = Z_all[0:32, :].rearrange("p (h x) -> p h x", h=2)
                for u in range(3):
                    zdst = zt[32 * u:32 * u + 32, :].rearrange(
                        "p (h x) -> p h x", h=2)
                    nc.gpsimd.dma_start(
                        zdst, zsrc[:, :, u * 196:(u + 1) * 196])
                wst["zt"] = zt

            def back2(wst):
                zt, w = wst["zt"], wst["w"]
                yp = p_psum.tile([128, 512], F32, tag="ps2")
                for nt in range(2):
                    for kt in range(2):
                        nc.tensor.matmul(
                            yp[0:98, nt * 256:nt * 256 + 192],
                            zt[:, kt * 196 + nt * 98: kt * 196 + nt * 98 + 98],
                            wp_t[kt][:, :],
                            start=(kt == 0), stop=(kt == 1),
                        )
                y_sb = y_pool.tile([98, 384], F32, tag="ysb")
                ysrc = yp[0:98, 0:512].rearrange("p (t x) -> p t x", t=2)[:, :, 0:192]
                ydst = y_sb[:, :].rearrange("p (t x) -> p t x", t=2)
                nc.scalar.copy(ydst, ysrc)
                nc.sync.dma_start(
                    out_d[w, :, :].rearrange("(t p) x -> p t x", t=2),
                    ydst)

            # prologue
            emit_xa(0)
            emit_qkproj(0)

            for it in range(NPAIR):
                if it + 1 < NPAIR:
                    emit_xa(it + 1)
                if len(stage1) > 3:  # pairs are pushed per window: 2/pair
                    back1(stage1.pop(0))
                    back1(stage1.pop(0))

                w0 = emit_vproj(it, 0)
                emit_s_exp(w0, 0)
                if len(stage2) > 5:
                    back2(stage2.pop(0))
                emit_s_exp(w0, 1)
                w1 = emit_vproj(it, 1)
                emit_pv(w0, 0)
                emit_pv(w0, 1)
                emit_u_recip(w0)
                emit_s_exp(w1, 0)
                if len(stage2) > 5:
                    back2(stage2.pop(0))
                emit_s_exp(w1, 1)
                if it + 1 < NPAIR:
                    emit_qkproj(it + 1)
                emit_pv(w1, 0)
                emit_pv(w1, 1)
                emit_u_recip(w1)

                stage1 += [w0, w1]
                stage2 += [w0, w1]

            while stage2:
                while stage1:
                    back1(stage1.pop(0))
                back2(stage2.pop(0))
    nc.compile()
    return nc


def _host_precompute(x, w_qkv, w_proj, bias_table, mask, rel_index):
    scale = HD ** (-0.5)
    wq = np.array(w_qkv, np.float32).copy()
    wq[0:C] *= scale  # fold softmax scale into q weights

    # xa[w, p, kt*196 + j] = x[w, j, kt*96 + p]; row 96: kt0->0, kt1->1
    xT = np.ascontiguousarray(np.transpose(np.asarray(x, np.float32), (0, 2, 1)))
    xa = np.zeros((B, 97, 392), np.float32)
    xa[:, 0:96, 0:196] = xT[:, 0:96]
    xa[:, 0:96, 196:392] = xT[:, 96:192]
    xa[:, 96, 196:392] = 1.0
    # pack window pairs: [B/2, 97, 784] = (w0 | w1)
    xa = np.ascontiguousarray(
        xa.reshape(B // 2, 2, 97, 392).transpose(0, 2, 1, 3).reshape(B // 2, 97, 784))

    # wqk[kt, p, f] = wq[f, kt*96+p]  (f < 384: q then k features)
    wqkT = wq[0:384].T  # [192, 384]
    wqk = np.stack([wqkT[0:96], wqkT[96:192]])

    # wv[kt, p, 33h+d] = wq[384+32h+d, kt*96+p]; ones row kt1 p=96
    wv = np.zeros((2, 97, 198), np.float32)
    wvT = wq[384:576].T  # [192, 192] [c, (h,d)]
    for h in range(H):
        wv[0, 0:96, 33 * h: 33 * h + 32] = wvT[0:96, 32 * h: 32 * h + 32]
        wv[1, 0:96, 33 * h: 33 * h + 32] = wvT[96:192, 32 * h: 32 * h + 32]
        wv[1, 96, 33 * h + 32] = 1.0

    # wp[kt]: rows in zt u-order (h0,h2,h1 | h3,h5,h4)
    wpT = np.asarray(w_proj, np.float32).T  # [c, c']
    perm = np.r_[0:32, 64:96, 32:64]
    wp = np.stack([wpT[0:96][perm], wpT[96:192][perm]])

    # EBM[w, p, h*392 + mt*196 + n] = exp(bias[n, m, h] + mask[w, n, m]),
    # m = mt*98 + p
    bias = np.asarray(bias_table, np.float32)[np.asarray(rel_index).reshape(-1)]
    bias = bias.reshape(N, N, H)  # [n, m, h]
    biasT = np.transpose(bias, (2, 1, 0))  # [h, m, n]
    maskT = np.transpose(np.asarray(mask, np.float32), (0, 2, 1))  # [g, m, n]
    ebm = np.exp(biasT[None] + maskT[:, None])  # [g, h, m, n]
    ebm = ebm.reshape(NG, H, 2, MT, N).transpose(0, 3, 1, 2, 4)
    ebm = np.ascontiguousarray(ebm.reshape(NG, MT, H * 392))

    return (xa.astype(NPBF16), wqk.astype(NPBF16), wv.astype(NPBF16),
            wp.astype(NPBF16), ebm.astype(NPBF16))


def kernel(x, w_qkv, w_proj, b_proj, bias_table, mask, rel_index):
    xa, wqk, wv, wp, ebm = _host_precompute(
        x, w_qkv, w_proj, bias_table, mask, rel_index)

    if "nc" not in _CACHE:
        _CACHE["nc"] = _build_nc()
    nc = _CACHE["nc"]

    in_maps = []
    for c in range(NCORES):
        in_maps.append({
            "xa": np.ascontiguousarray(xa[c * NPAIR:(c + 1) * NPAIR]),
            "ebm": ebm,  # window w on core uses mask (64c+w) % 64 = w
            "wqk": wqk, "wv": wv, "wp": wp,
        })

    res = bass_utils.run_bass_kernel_spmd(nc, in_maps, core_ids=list(range(NCORES)))
    out = np.concatenate([res.results[c]["out"] for c in range(NCORES)], axis=0)
    out = out.astype(np.float32) + np.asarray(b_proj, np.float32)[None, None, :]
    return out
